# revision 1
# baseline (speedup 1.0000x reference)
"""TRN2 Bass kernel for nn_COV_75359496176097.

reference():
    B2 = B[0]                               # (8192, 8192)
    rn = sqrt(1 / sum(B2*B2, axis=1))       # row norms
    A  = rn * B2 * exp(tile(logstd, 64))[:, None]
    samples = tile(mu,64) + einsum('mk,bk->bm', A, eps[:,:,0])
    returns (mu_out, logvar, samples), each (128, 64, 128)

Strategy: shard B by rows across 8 cores (1024 rows each, no collectives).
Each core computes out[b, r] = sum_k eps[k, b] * B[r, k] on the PE
(eps k-tile stationary fp32r, B^T k-tile moving fp32r, PSUM-accumulated
over 64 k-tiles; fp32r streams at full fp32-ish precision, measured
~8e-5 max rel err). Row norms ride along: DVE squares each B^T tile to
bf16 and an all-ones bf16 stationary matmul accumulates the column sums
into a second PSUM bank — replicating them across all 128 output
partitions for free, and doubling as pipeline shadow for the fp32 weight
loads of the fp32r matmuls. A 24-matmul warmup keeps the PE's HAM clock
monitor in the full-speed state before the first B tile lands. Epilogue:
out = (acc*els) * 1/sqrt(nrm) + mu with acc*els overlapped into the loop
tail, ACT sqrt, and a two-op ~2ULP approximate reciprocal on DVE.

Raw Bass (not Tile): hardware allows at most ONE semaphore wait per
instruction, and this dataflow (each DMA'd tile consumed by PE and DVE)
needs transitive cross-engine reasoning Tile doesn't do. Manual scheme:
per-slot DMA-completion semaphores; PE's norm matmul for tile t waits on
DVE's square, so "PE retired tile t" implies every consumer of slot t is
done; the DMA issuer throttles on that single PE semaphore.

Each k-tile's B^T slice and eps^T slice are packed side by side in one
host-prepared tensor so a k-tile needs exactly one DMA.
"""

import sys
from contextlib import ExitStack

if "/opt/trn_rl_repo" not in sys.path:
    sys.path.insert(0, "/opt/trn_rl_repo")

import numpy as np

import concourse.bacc as bacc
import concourse.mybir as mybir
from concourse import bass_utils
from concourse.dve_ops import RECIPROCAL_APPROX_NR

Z = 128
NS = 64
M = Z * NS          # 8192
BATCH = 128
NCORES = 8
RPC = M // NCORES   # 1024 rows of B per core
KT = M // 128       # 64 k-tiles
W = RPC + BATCH     # 1152 packed row width
NB = 14             # B-tile SBUF slots (DMA prefetch depth)
SPLITS = {0: 4, 1: 4, 2: 4, 3: 4}  # first tiles DMA'd in chunks (parallel ramp-up)
EXTRA = {s: 16 * (n - 1) for s, n in SPLITS.items()}

f32 = mybir.dt.float32
f32r = mybir.dt.float32r
bf16 = mybir.dt.bfloat16

_nc_cache = {}


def _dma_need(t):
    """semaphore threshold for tile t's slot DMA(s) to have completed"""
    return 16 * (t // NB + 1) + EXTRA.get(t % NB, 0)


def _build():
    nc = bacc.Bacc("TRN2", debug=False)

    bte_d = nc.dram_tensor("bte", (M, W), f32r, kind="ExternalInput")
    els_d = nc.dram_tensor("els", (BATCH, RPC), f32, kind="ExternalInput")
    mu_d = nc.dram_tensor("mu", (BATCH, RPC), f32, kind="ExternalInput")
    out_d = nc.dram_tensor("out", (BATCH, RPC), f32, kind="ExternalOutput")

    with ExitStack() as ctx:
        e = ctx.enter_context
        slots = [e(nc.sbuf_tensor(f"slot{i}", [128, W], f32r)) for i in range(NB)]
        sq = [e(nc.sbuf_tensor(f"sq{i}", [128, RPC], bf16)) for i in range(NB)]
        ones = e(nc.sbuf_tensor("ones", [128, 128], bf16))
        els_sb = e(nc.sbuf_tensor("els_sb", [128, RPC], f32))
        mu_sb = e(nc.sbuf_tensor("mu_sb", [128, RPC], f32))
        inv_sb = e(nc.sbuf_tensor("inv_sb", [128, RPC], f32))
        rn_sb = e(nc.sbuf_tensor("rn_sb", [128, RPC], f32))
        scale_sb = e(nc.sbuf_tensor("scale_sb", [128, RPC], f32))
        out_sb = e(nc.sbuf_tensor("out_sb", [128, RPC], f32))
        acc = e(nc.psum_tensor([128, RPC], f32))
        nrm = e(nc.psum_tensor([128, RPC], f32))
        warm_ps = e(nc.psum_tensor([128, 128], f32))

        s_dma = [e(nc.semaphore(name=f"s_dma{i}")) for i in range(NB)]
        s_cst = e(nc.semaphore(name="s_cst"))
        s_pe = e(nc.semaphore(name="s_pe"))
        s_dve = e(nc.semaphore(name="s_dve"))
        s_act = e(nc.semaphore(name="s_act"))
        s_x = e(nc.semaphore(name="s_x"))
        s_acc = e(nc.semaphore(name="s_acc"))
        s_wm = e(nc.semaphore(name="s_wm"))
        s_ep = e(nc.semaphore(name="s_ep"))
        s_out = e(nc.semaphore(name="s_out"))
        s_od = e(nc.semaphore(name="s_od"))

        block = e(nc.Block())

        @block.sync
        def _(sync):
            for t in range(KT):
                sl = slice(t * 128, (t + 1) * 128)
                if t == NB:
                    # constants only needed by the epilogue; issue after the
                    # first wave of B-tile DMAs so the PE starts sooner
                    sync.dma_start(els_sb[:], els_d.ap()[:, :]).then_inc(
                        s_cst, 16
                    )
                    sync.dma_start(mu_sb[:], mu_d.ap()[:, :]).then_inc(
                        s_cst, 16
                    )
                if t >= NB:
                    # slot free once PE's norm matmul of tile t-NB retired
                    # (transitively implies DVE's square is done too)
                    sync.wait_ge(s_pe, t - NB + 1)
                if t < NB and t % 2 == 1:
                    continue  # odd burst tiles ride ACT's HWDGE queue
                nchunk = SPLITS.get(t, 1)
                p = 128 // nchunk
                for ci in range(nchunk):
                    sync.dma_start(
                        slots[t % NB][ci * p:(ci + 1) * p, :],
                        bte_d.ap()[sl, :][ci * p:(ci + 1) * p, :],
                    ).then_inc(s_dma[t % NB], 16)
            for h in range(2):
                hs = slice(h * 512, (h + 1) * 512)
                sync.wait_ge(s_out, h + 1)
                for ci in range(2):
                    ps = slice(ci * 64, (ci + 1) * 64)
                    sync.dma_start(
                        out_d.ap()[ps, hs], out_sb[ps, hs]
                    ).then_inc(s_od, 16)
            sync.wait_ge(s_od, 64)
            sync.nop()

        @block.tensor
        def _(tensor):
            # warmup matmuls: pin the PE HAM activity monitor to the warm
            # (full-clock) state before the first B tile lands
            tensor.wait_ge(s_wm, 1)
            for _ in range(40):
                nc.tensor.matmul(
                    warm_ps[:, 0:128], ones[:], ones[:], start=True, stop=True
                )

            def norm_mms(tensor, j):
                # norm matmuls run one tile behind the acc matmuls so the
                # square producers (DVE h0 / ACT h1) never stall the PE;
                # they also double as LDW shadow for the fp32r pairs
                sj = j % NB
                jst, jsp = j == 0, j == KT - 1
                tensor.wait_ge(s_dve, j + 1)
                nc.tensor.matmul(
                    nrm[:, 0:512], ones[:], sq[sj][:, 0:512],
                    start=jst, stop=jsp,
                )
                tensor.wait_ge(s_act, j + 1)
                return nc.tensor.matmul(
                    nrm[:, 512:RPC], ones[:], sq[sj][:, 512:RPC],
                    start=jst, stop=jsp,
                ).then_inc(s_pe, 1)

            for t in range(KT):
                st, sp = t == 0, t == KT - 1
                s = t % NB
                tensor.wait_ge(s_dma[s], _dma_need(t))
                eps_v = slots[s][:, RPC:W]
                for h in range(RPC // 512):
                    hs = slice(h * 512, (h + 1) * 512)
                    ins = nc.tensor.matmul(
                        acc[:, hs], eps_v, slots[s][:, hs], start=st, stop=sp
                    )
                if sp:
                    # lets DVE start acc*els while the norm matmuls finish
                    ins.then_inc(s_acc, 1)
                if t >= 1:
                    norm_mms(tensor, t - 1)
            norm_mms(tensor, KT - 1)

        @block.scalar
        def _(scalar):
            for t in range(1, NB, 2):
                sl = slice(t * 128, (t + 1) * 128)
                nchunk = SPLITS.get(t, 1)
                p = 128 // nchunk
                for ci in range(nchunk):
                    scalar.dma_start(
                        slots[t % NB][ci * p:(ci + 1) * p, :],
                        bte_d.ap()[sl, :][ci * p:(ci + 1) * p, :],
                    ).then_inc(s_dma[t % NB], 16)
            for t in range(KT):
                s = t % NB
                scalar.wait_ge(s_dma[s], _dma_need(t))
                nc.scalar.square(
                    sq[s][:, 512:RPC], slots[s][:, 512:RPC].bitcast(f32)
                ).then_inc(s_act, 1)
            scalar.wait_ge(s_pe, KT)
            nc.scalar.sqrt(inv_sb[:, 0:512], nrm[:, 0:512]).then_inc(s_x, 1)
            nc.scalar.sqrt(inv_sb[:, 512:RPC], nrm[:, 512:RPC]).then_inc(
                s_x, 1
            )

        @block.vector
        def _(vector):
            nc.vector.memset(ones[:], 1.0).then_inc(s_wm, 1)
            for t in range(KT):
                s = t % NB
                # the slot DMA only fired after PE retired tile t-NB, so the
                # sq[s] anti-dependency (PE read of square t-NB) is implied
                vector.wait_ge(s_dma[s], _dma_need(t))
                btf = slots[s][:, 0:512].bitcast(f32)
                nc.vector.tensor_mul(
                    sq[s][:, 0:512], btf, btf
                ).then_inc(s_dve, 1)
            # epilogue: out = (acc*els) / sqrt(nrm) + mu, pipelined by
            # column halves.  Dependent same-half ops are distance-2 in the
            # stream; s_ep self-waits (satisfied at producer retirement)
            # replace full-pipeline drains.  acc*els overlaps the final norm
            # matmuls and the ACT sqrt.
            H = (slice(0, 512), slice(512, RPC))
            vector.wait_ge(s_cst, 32)
            vector.nop()
            vector.wait_ge(s_acc, 1)
            nc.vector.tensor_mul(
                scale_sb[:, H[0]], acc[:, H[0]], els_sb[:, H[0]]
            ).then_inc(s_ep, 1)  # e1
            nc.vector.tensor_mul(
                scale_sb[:, H[1]], acc[:, H[1]], els_sb[:, H[1]]
            ).then_inc(s_ep, 1)  # e2
            for h in (0, 1):  # e3, e4: recip seed of sqrt(nrm)
                vector.wait_ge(s_x, h + 1)
                nc.vector.reciprocal_approx_fast(
                    out=rn_sb[:, H[h]], in_=inv_sb[:, H[h]]
                ).then_inc(s_ep, 1)
            for h in (0, 1):  # e5, e6: Newton-Raphson refine -> out_sb
                vector.wait_ge(s_ep, 3 + h)
                nc.vector._custom_dve(
                    RECIPROCAL_APPROX_NR,
                    out=out_sb[:, H[h]],
                    in0=inv_sb[:, H[h]],
                    in1=rn_sb[:, H[h]],
                    s0=2.0,
                ).then_inc(s_ep, 1)
            for h in (0, 1):  # e7, e8: * (acc*els)
                vector.wait_ge(s_ep, 5 + h)
                nc.vector.tensor_mul(
                    out_sb[:, H[h]], scale_sb[:, H[h]], out_sb[:, H[h]]
                ).then_inc(s_ep, 1)
            for h in (0, 1):  # e9, e10: + mu, releases the half's out DMA
                vector.wait_ge(s_ep, 7 + h)
                nc.vector.tensor_add(
                    out_sb[:, H[h]], out_sb[:, H[h]], mu_sb[:, H[h]]
                ).then_inc(s_out, 1)

    nc.compile()
    return nc


def _get_nc():
    if "nc" not in _nc_cache:
        _nc_cache["nc"] = _build()
    return _nc_cache["nc"]


def _prep_inputs(mu, logstd, B, eps):
    B2 = B[0]
    epst = np.ascontiguousarray(eps[:, :, 0].T)        # (M, BATCH)
    mu_rep = np.tile(mu[0], NS)                        # (M,)
    logstd_rep = np.tile(logstd, NS)                   # (M,)
    els_rep = np.exp(logstd_rep).astype(np.float32)    # (M,)

    in_maps = []
    for c in range(NCORES):
        rows = slice(c * RPC, (c + 1) * RPC)
        bte = np.empty((M, W), dtype=np.float32)
        bte[:, 0:RPC] = B2[rows, :].T
        bte[:, RPC:W] = epst
        in_maps.append(
            {
                "bte": bte,
                "els": np.ascontiguousarray(
                    np.broadcast_to(els_rep[rows][None, :], (BATCH, RPC))
                ),
                "mu": np.ascontiguousarray(
                    np.broadcast_to(mu_rep[rows][None, :], (BATCH, RPC))
                ),
            }
        )
    return in_maps, mu_rep, logstd_rep


def _run(mu, logstd, B, eps, batch_size, trace=False, trace_kwargs=None):
    mu = np.asarray(mu, dtype=np.float32)
    logstd = np.asarray(logstd, dtype=np.float32)
    B = np.asarray(B, dtype=np.float32)
    eps = np.asarray(eps, dtype=np.float32)
    b = int(batch_size)
    assert B.shape == (1, M, M) and eps.shape == (b, M, 1) and b == BATCH

    in_maps, mu_rep, logstd_rep = _prep_inputs(mu, logstd, B, eps)

    nc = _get_nc()
    kw = {}
    if trace:
        kw = dict(trace=True, trace_cores=list(range(NCORES)))
        if trace_kwargs:
            kw.update(trace_kwargs)
    res = bass_utils.run_bass_kernel_spmd(
        nc, in_maps, core_ids=list(range(NCORES)), **kw
    )

    samples_bm = np.concatenate(
        [res.results[c]["out"] for c in range(NCORES)], axis=1
    )  # (BATCH, M)
    samples = samples_bm.reshape(b, NS, Z)
    mu_out = np.broadcast_to(mu_rep[None, :], (b, M)).reshape(b, NS, Z).copy()
    logvar = (
        np.broadcast_to(2.0 * logstd_rep[None, :], (b, M)).reshape(b, NS, Z).copy()
    )
    return (mu_out, logvar, samples), res


def kernel(mu, logstd, B, eps, batch_size):
    outs, _ = _run(mu, logstd, B, eps, batch_size, trace=False)
    return outs



# revision 3
# speedup vs baseline: 1.6452x; 1.6452x over previous
"""TRN2 Bass kernel for nn_COV_75359496176097.

reference():
    B2 = B[0]                               # (8192, 8192)
    rn = sqrt(1 / sum(B2*B2, axis=1))       # row norms
    A  = rn * B2 * exp(tile(logstd, 64))[:, None]
    samples = tile(mu,64) + einsum('mk,bk->bm', A, eps[:,:,0])
    returns (mu_out, logvar, samples), each (128, 64, 128)

Strategy: shard B by rows across 8 cores (1024 rows each, no collectives),
all data in bf16 (tolerance 2e-2 >> bf16's ~3e-3 dot error), which halves
HBM traffic vs fp32 — per core ~19.4MB -> ~54us at the 358 GB/s/core HBM
roofline. The whole packed input (64 k-tiles of [B^T | eps^T] plus
replicated exp(logstd) and mu rows, 148KB/partition) fits in SBUF, so
there is no slot recycling: 15 large FIFO-ordered DMAs stream on the sync
HWDGE queue (big chunks mid-stream for efficiency, small tail chunks so
the final tiles' compute starts as early as possible), each with its own
completion semaphore.

Per k-tile the PE runs two bf16 acc matmuls (eps^T stationary, B^T
moving, PSUM-accumulated). Row norms: DVE (cols 0:544) and ACT (544:1024)
square each B^T tile to bf16; DVE then sums square-tiles pairwise so the
ones-stationary norm matmul runs once per PAIR of tiles - halving that
matmul's moving-column cost (PE ~41us total, under the DMA roofline).
Epilogue: out = (acc*els) * sqrt(1/nrm) + mu using DVE
reciprocal_approx_fast on PSUM nrm, ACT sqrt (Square and Sqrt live in the
same ACT table set - a dummy sqrt at warmup loads it once), then DVE
mul/add per column half. A 40-matmul warmup keeps the PE HAM clock
monitor warm until the first tile lands.

Raw Bass (not Tile): hardware allows at most ONE semaphore wait per
instruction; consecutive standalone waits AND together. Norm matmuls run
one pair behind the square producers so DVE/ACT never stall the PE.
"""

import sys
from contextlib import ExitStack

if "/opt/trn_rl_repo" not in sys.path:
    sys.path.insert(0, "/opt/trn_rl_repo")

import ml_dtypes
import numpy as np

import concourse.bacc as bacc
import concourse.mybir as mybir
from concourse import bass_utils

Z = 128
NS = 64
M = Z * NS          # 8192
BATCH = 128
NCORES = 8
RPC = M // NCORES   # 1024 rows of B per core
KT = M // 128       # 64 k-tiles
TW = RPC + BATCH    # 1152 packed tile width (B^T cols | eps^T cols)
TCOLS = KT * TW     # 73728 tile columns
NTC = TCOLS + 2 * RPC  # + els, mu replicated rows -> 75776 total sbuf cols
WD = 544            # DVE squares cols [0:544), ACT squares [544:1024)
NPAIR = KT // 2

# DMA chunks, in tiles; 'EM' is the els/mu constants chunk. Front chunks
# small (start compute early), middle big (DMA efficiency), tail small
# (minimize serial work after the last byte lands).
CHUNKS = [
    (0, 2), (2, 4), (4, 8), "EM",
    (8, 16), (16, 24), (24, 32), (32, 40), (40, 48),
    (48, 54), (54, 58), (58, 60), (60, 62), (62, 63), (63, 64),
]
NDMA = len(CHUNKS)
EM_IDX = CHUNKS.index("EM")


def _tile_dma_idx(t):
    for i, ch in enumerate(CHUNKS):
        if ch == "EM":
            continue
        if ch[0] <= t < ch[1]:
            return i
    raise AssertionError(t)


f32 = mybir.dt.float32
bf16 = mybir.dt.bfloat16

_nc_cache = {}


def _build():
    nc = bacc.Bacc("TRN2", debug=False)

    bte_d = nc.dram_tensor("bte", (128, NTC), bf16, kind="ExternalInput")
    out_d = nc.dram_tensor("out", (BATCH, RPC), f32, kind="ExternalOutput")

    with ExitStack() as ctx:
        e = ctx.enter_context
        bte = e(nc.sbuf_tensor("bte_sb", [128, NTC], bf16))
        sqe = [e(nc.sbuf_tensor(f"sqe{i}", [128, RPC], bf16)) for i in range(2)]
        sqo = [e(nc.sbuf_tensor(f"sqo{i}", [128, RPC], bf16)) for i in range(2)]
        ones = e(nc.sbuf_tensor("ones", [128, 128], bf16))
        dummy = e(nc.sbuf_tensor("dmy_sb", [128, 1], f32))
        scale = e(nc.sbuf_tensor("scale", [128, RPC], f32))
        rr = e(nc.sbuf_tensor("rr", [128, RPC], f32))
        rs = e(nc.sbuf_tensor("rs", [128, RPC], f32))
        out_sb = e(nc.sbuf_tensor("out_sb", [128, RPC], f32))
        acc = e(nc.psum_tensor([128, RPC], f32))
        nrm = e(nc.psum_tensor([128, RPC], f32))
        warm_ps = e(nc.psum_tensor([128, 512], f32))

        s_dma = [e(nc.semaphore(name=f"s_dma{i}")) for i in range(NDMA)]
        s_wm = e(nc.semaphore(name="s_wm"))
        s_act = e(nc.semaphore(name="s_act"))
        s_d0 = e(nc.semaphore(name="s_d0"))
        s_d1 = e(nc.semaphore(name="s_d1"))
        s_nm = e(nc.semaphore(name="s_nm"))
        s_acc = e(nc.semaphore(name="s_acc"))
        s_r = e(nc.semaphore(name="s_r"))
        s_x = e(nc.semaphore(name="s_x"))
        s_out = e(nc.semaphore(name="s_out"))
        s_od = e(nc.semaphore(name="s_od"))

        block = e(nc.Block())

        H = (slice(0, 512), slice(512, RPC))

        def bslice(t, a, b):
            return bte[:, t * TW + a:t * TW + b]

        @block.sync
        def _(sync):
            for i, ch in enumerate(CHUNKS):
                if ch == "EM":
                    c0, c1 = TCOLS, NTC
                else:
                    c0, c1 = ch[0] * TW, ch[1] * TW
                sync.dma_start(bte[:, c0:c1], bte_d.ap()[:, c0:c1]).then_inc(
                    s_dma[i], 16
                )
            for h in range(2):
                sync.wait_ge(s_out, h + 1)
                sync.dma_start(out_d.ap()[:, H[h]], out_sb[:, H[h]]).then_inc(
                    s_od, 16
                )
            sync.wait_ge(s_od, 32)
            sync.nop()

        @block.tensor
        def _(tensor):
            # warmup: pin the PE HAM activity monitor warm until tile 0 lands
            tensor.wait_ge(s_wm, 1)
            for _ in range(40):
                nc.tensor.matmul(
                    warm_ps[:, 0:128], ones[:], ones[:], start=True, stop=True
                )

            def norm_mms(q):
                qst, qsp = q == 0, q == NPAIR - 1
                tensor.wait_ge(s_d0, q + 1)
                nc.tensor.matmul(
                    nrm[:, H[0]], ones[:], sqe[q % 2][:, H[0]],
                    start=qst, stop=qsp,
                )
                tensor.wait_ge(s_d1, q + 1)
                nc.tensor.matmul(
                    nrm[:, H[1]], ones[:], sqe[q % 2][:, H[1]],
                    start=qst, stop=qsp,
                ).then_inc(s_nm, 1)

            seen = -1
            for t in range(KT):
                di = _tile_dma_idx(t)
                if di > seen:
                    tensor.wait_ge(s_dma[di], 16)
                    seen = di
                st, sp = t == 0, t == KT - 1
                eps_v = bslice(t, RPC, TW)
                for h in range(2):
                    ins = nc.tensor.matmul(
                        acc[:, H[h]], eps_v, bslice(t, h * 512, (h + 1) * 512),
                        start=st, stop=sp,
                    )
                if sp:
                    ins.then_inc(s_acc, 1)
                if t % 2 == 1 and t >= 3:
                    # norm matmuls run one pair behind the square producers
                    norm_mms(t // 2 - 1)
            norm_mms(NPAIR - 1)

        @block.scalar
        def _(scalar):
            # dummy sqrt first: loads the sqrt_and_others ACT table set
            # (which also contains Square) once, during the DMA fill
            scalar.wait_ge(s_wm, 1)
            nc.scalar.sqrt(dummy[:], ones[:, 0:1])
            seen = -1
            for p in range(NPAIR):
                te, to = 2 * p, 2 * p + 1
                die, dio = _tile_dma_idx(te), _tile_dma_idx(to)
                if die > seen:
                    scalar.wait_ge(s_dma[die], 16)
                    seen = die
                if p >= 2:
                    scalar.wait_ge(s_nm, p - 1)
                nc.scalar.square(sqe[p % 2][:, WD:RPC], bslice(te, WD, RPC))
                if dio > seen:
                    scalar.wait_ge(s_dma[dio], 16)
                    seen = dio
                nc.scalar.square(
                    sqo[p % 2][:, WD:RPC], bslice(to, WD, RPC)
                ).then_inc(s_act, 1)
            for h in range(2):
                scalar.wait_ge(s_r, h + 1)
                nc.scalar.sqrt(rs[:, H[h]], rr[:, H[h]]).then_inc(s_x, 1)

        @block.vector
        def _(vector):
            nc.vector.memset(ones[:], 1.0).then_inc(s_wm, 1)
            seen = -1
            for p in range(NPAIR):
                te, to = 2 * p, 2 * p + 1
                die, dio = _tile_dma_idx(te), _tile_dma_idx(to)
                if die > seen:
                    vector.wait_ge(s_dma[die], 16)
                    seen = die
                if p >= 2:
                    vector.wait_ge(s_nm, p - 1)
                be = bslice(te, 0, WD)
                nc.vector.tensor_mul(sqe[p % 2][:, 0:WD], be, be)
                if dio > seen:
                    vector.wait_ge(s_dma[dio], 16)
                    seen = dio
                bo = bslice(to, 0, WD)
                nc.vector.tensor_mul(sqo[p % 2][:, 0:WD], bo, bo)
                nc.vector.tensor_add(
                    sqe[p % 2][:, H[0]], sqe[p % 2][:, H[0]], sqo[p % 2][:, H[0]]
                ).then_inc(s_d0, 1)
                vector.wait_ge(s_act, p + 1)
                nc.vector.tensor_add(
                    sqe[p % 2][:, H[1]], sqe[p % 2][:, H[1]], sqo[p % 2][:, H[1]]
                ).then_inc(s_d1, 1)

            # epilogue: out = (acc*els) * sqrt(1/nrm) + mu, by column halves
            els = bte[:, TCOLS:TCOLS + RPC]
            mu_v = bte[:, TCOLS + RPC:NTC]
            vector.wait_ge(s_dma[EM_IDX], 16)
            vector.wait_ge(s_acc, 1)
            for h in range(2):
                nc.vector.tensor_mul(scale[:, H[h]], acc[:, H[h]], els[:, H[h]])
            vector.wait_ge(s_nm, NPAIR)
            for h in range(2):
                nc.vector.reciprocal_approx_fast(
                    out=rr[:, H[h]], in_=nrm[:, H[h]]
                ).then_inc(s_r, 1)
            for h in range(2):
                vector.wait_ge(s_x, h + 1)
                nc.vector.tensor_mul(out_sb[:, H[h]], scale[:, H[h]], rs[:, H[h]])
                nc.vector.tensor_add(
                    out_sb[:, H[h]], out_sb[:, H[h]], mu_v[:, H[h]]
                ).then_inc(s_out, 1)

    nc.compile()
    return nc


def _get_nc():
    if "nc" not in _nc_cache:
        _nc_cache["nc"] = _build()
    return _nc_cache["nc"]


def _prep_inputs(mu, logstd, B, eps):
    bfl = ml_dtypes.bfloat16
    B2 = B[0]
    Bb = B2.astype(bfl)                                  # (M, M)
    epsT3 = np.ascontiguousarray(eps[:, :, 0].T).astype(bfl).reshape(KT, 128, BATCH)
    mu_rep = np.tile(mu[0], NS)                          # (M,)
    logstd_rep = np.tile(logstd, NS)                     # (M,)
    els_rep = np.exp(logstd_rep).astype(np.float32)      # (M,)

    in_maps = []
    for c in range(NCORES):
        rows = slice(c * RPC, (c + 1) * RPC)
        bt3 = np.ascontiguousarray(Bb[rows, :].T).reshape(KT, 128, RPC)
        tile_block = np.concatenate([bt3, epsT3], axis=2)   # (KT, 128, TW)
        arr = np.empty((128, NTC), dtype=bfl)
        arr[:, 0:TCOLS] = tile_block.transpose(1, 0, 2).reshape(128, TCOLS)
        arr[:, TCOLS:TCOLS + RPC] = np.broadcast_to(
            els_rep[rows].astype(bfl)[None, :], (128, RPC)
        )
        arr[:, TCOLS + RPC:NTC] = np.broadcast_to(
            mu_rep[rows].astype(bfl)[None, :], (128, RPC)
        )
        in_maps.append({"bte": arr})
    return in_maps, mu_rep, logstd_rep


def _run(mu, logstd, B, eps, batch_size, trace=False, trace_kwargs=None):
    mu = np.asarray(mu, dtype=np.float32)
    logstd = np.asarray(logstd, dtype=np.float32)
    B = np.asarray(B, dtype=np.float32)
    eps = np.asarray(eps, dtype=np.float32)
    b = int(batch_size)
    assert B.shape == (1, M, M) and eps.shape == (b, M, 1) and b == BATCH

    in_maps, mu_rep, logstd_rep = _prep_inputs(mu, logstd, B, eps)

    nc = _get_nc()
    kw = {}
    if trace:
        kw = dict(trace=True, trace_cores=list(range(NCORES)))
        if trace_kwargs:
            kw.update(trace_kwargs)
    res = bass_utils.run_bass_kernel_spmd(
        nc, in_maps, core_ids=list(range(NCORES)), **kw
    )

    samples_bm = np.concatenate(
        [np.asarray(res.results[c]["out"], dtype=np.float32) for c in range(NCORES)],
        axis=1,
    )  # (BATCH, M)
    samples = samples_bm.reshape(b, NS, Z)
    mu_out = np.broadcast_to(mu_rep[None, :], (b, M)).reshape(b, NS, Z).copy()
    logvar = (
        np.broadcast_to(2.0 * logstd_rep[None, :], (b, M)).reshape(b, NS, Z).copy()
    )
    return (mu_out, logvar, samples), res


def kernel(mu, logstd, B, eps, batch_size):
    outs, _ = _run(mu, logstd, B, eps, batch_size, trace=False)
    return outs
